# revision 19
# baseline (speedup 1.0000x reference)
"""V4: routed kernel, all-bf16, ragged per-expert capacities, balanced cores.

Data-parallel over 8 cores, weights replicated (bf16).  Rows are assigned to
cores by a balancing pass so each (core, module-type, expert) group fits a
capacity close to ceil(global_count/8); capacities are per-(type, expert)
program constants (ragged), cutting padding vs a uniform worst-case C.

- All matmul operands are bf16 (fp32 PSUM accumulate): first layers use the
  order-agnostic expanded-input trick (K=128), big layers run routed
  (4x fewer FLOPs than dense).
- L0_1/L1_1/L2_1 run operand-swapped (lhsT = activation column block, rhs =
  that group's expert weights), PSUM output batch-major; ReLU evacuation
  writes a chunk-major token tile which a native indirect-scatter DMA writes
  to DRAM rows in the NEXT stage's expert order, and XBAR transpose-DMAs
  load it back feature-major.
- L3_1 and the head run feature-major grouped (no transition after them).
- Biases: ACT bias operand (feature-major layers, exact fp32) or a K=1
  ones-outer-product matmul into PSUM (batch-major layers, bf16 bias).
- Per-expert weight K-stacks load as ONE strided DMA per (layer, expert).
"""

import numpy as np
import ml_dtypes
from contextlib import ExitStack

import concourse.bass as bass
import concourse.bacc as bacc
import concourse.tile as tile
import concourse.mybir as mybir
from concourse import bass_utils

F32 = mybir.dt.float32
BF16 = mybir.dt.bfloat16
I32 = mybir.dt.int32
RELU = mybir.ActivationFunctionType.Relu
COPY = mybir.ActivationFunctionType.Copy

B = 8192
NCORES = 8
BC = B // NCORES
FEAT = 32
M = 4
H = 512
OUT = 8
P = 128
KBIG = [4, 8, 8, 8]
ABLATE = set()


def _chunks(caps):
    """Chunk split of ragged groups: [(m, g0, r)] with r<=128."""
    out = []
    off0 = 0
    for m in range(M):
        off = 0
        while off < caps[m]:
            r = min(P, caps[m] - off)
            out.append((m, off0 + off, r))
            off += r
        off0 += caps[m]
    return out


def _offs(caps):
    o, s = [], 0
    for c in caps:
        o.append(s)
        s += c
    return o, s


def _emit(nc, tc, ctx, d, caps):
    capmax = max(max(c) for c in caps)
    offs, bps = zip(*[_offs(c) for c in caps])        # per-type offsets, Bp
    chunks = [_chunks(c) for c in caps]               # per-type chunk lists
    nch = [len(ch) for ch in chunks]
    tboff, _ = _offs(nch[:3])

    consts = ctx.enter_context(tc.tile_pool(name="consts", bufs=1))
    wpool = ctx.enter_context(tc.tile_pool(name="wbig", bufs=4))
    hp = ctx.enter_context(tc.tile_pool(name="hacts", bufs=1))
    permp = ctx.enter_context(tc.tile_pool(name="perm", bufs=1))
    outp = ctx.enter_context(tc.tile_pool(name="outs", bufs=1))
    psp = ctx.enter_context(tc.tile_pool(name="psum", bufs=8, space="PSUM"))

    # ---------------- constants ----------------
    xe_t = []
    for j in range(4):
        t = consts.tile([P, bps[j]], BF16, tag=f"xe{j}", name=f"xe{j}")
        nc.sync.dma_start(t[:], d[f"xe{j}"].ap())
        xe_t.append(t)
    wf_t = []
    for j in range(4):
        t = consts.tile([P, H], BF16, tag=f"wf{j}", name=f"wf{j}")
        nc.sync.dma_start(t[:], d["Wf"].ap()[j, :, :])
        wf_t.append(t)
    w32_t = []
    for m in range(M):
        t = consts.tile([P, 4, OUT], BF16, tag=f"w32_{m}", name=f"w32_{m}")
        nc.sync.dma_start(
            t[:], d["W32"].ap()[m, :, :].rearrange("(a p) o -> p a o", p=P))
        w32_t.append(t)
    bias_sb = consts.tile([P, 8 * 16], F32, tag="bias", name="bias")
    nc.sync.dma_start(bias_sb[:], d["bias"].ap())
    bh = consts.tile([OUT, 4], F32, tag="bh", name="bh")
    nc.sync.dma_start(bh[:], d["bh"].ap())
    brow = consts.tile([1, 12, H], BF16, tag="brow", name="brow")
    nc.gpsimd.dma_start(brow[:], d["brow"].ap())
    ones = consts.tile([1, P], BF16, tag="ones", name="ones")
    nc.sync.dma_start(ones[:], d["ones"].ap())
    tbl = consts.tile([P, sum(nch[:3])], I32, tag="tbl", name="tbl")
    nc.sync.dma_start(tbl[:], d["tbl"].ap())
    ident = consts.tile([P, P], BF16, tag="ident", name="ident")
    nc.sync.dma_start(ident[:], d["ident"].ap())

    def bias_ap(layer, hh, m):
        col = layer * 16 + hh * 4 + m
        return bias_sb[:, col:col + 1]

    # ---------------- layers ----------------
    def first_layer(j, tag):
        """relu(Wf[j].T @ xe_g[j] + b_j0): 4x [128, Bp_j] bf16, feat-major."""
        outs = []
        for hpair in range(2):
            ps = [[psp.tile([P, capmax], F32, tag="pt", name="pt")
                   for m in range(M)] for _ in range(2)]
            for hi in range(2):
                hh = hpair * 2 + hi
                for m in range(M):
                    o, c = offs[j][m], caps[j][m]
                    nc.tensor.matmul(
                        ps[hi][m][:, :c], wf_t[j][:, bass.ts(hh, P)],
                        xe_t[j][:, o:o + c], start=True, stop=True)
            for hi in range(2):
                hh = hpair * 2 + hi
                t = hp.tile([P, bps[j]], BF16, tag=f"{tag}{hh}",
                            name=f"{tag}{hh}")
                for m in range(M):
                    o, c = offs[j][m], caps[j][m]
                    nc.scalar.activation(t[:, o:o + c], ps[hi][m][:, :c],
                                         RELU, bias=bias_ap(2 * j, hh, m))
                outs.append(t)
        return outs

    def load_w(j, m, Kc):
        w = wpool.tile([P, Kc, H], BF16, tag="wt", name="wt")
        if "noweights" in ABLATE:
            return [w[:, k, :] for k in range(Kc)]
        nc.sync.dma_start(
            w[:], d[f"W{j}1"].ap()[m, :, :].rearrange("(k p) h -> p k h", p=P))
        return [w[:, k, :] for k in range(Kc)]

    def _evac_relu(ch, dst, src):
        if "noevac" in ABLATE:
            return
        if ch % 2:
            nc.scalar.activation(dst, src, RELU)
        else:
            nc.vector.tensor_relu(dst, src)

    def swapped_big(j, z_zx, z_h, nA):
        """relu(W_j1[expert].T @ z + b), batch-major out -> xsc token tile.

        Chunks [0:nA) run two-pass: bias + first-layer-feature half first
        (independent of the inter-layer transition), zx half after -- PSUM
        holds the partials so PE has work while the transition DMAs run."""
        Kc = KBIG[j]
        nzx = len(z_zx)
        xsc = permp.tile([P, nch[j], H], BF16, tag="xsc", name="xsc", bufs=2)
        ws_all = [load_w(j, m, Kc) for m in range(M)]
        pbs = {}

        def bias_h(ch, m, g0, r):
            pb = psp.tile([P, H], F32, tag="pt", name="pt")
            pbs[ch] = pb
            lm = 4 * j + m
            nc.tensor.matmul(pb[:r, :], ones[:, :r],
                             brow[:, lm, :],
                             start=True, stop=False)
            zh = z_h if nzx else z_h[:-1]
            if "nomm" in ABLATE:
                zh = []
            for i, zt in enumerate(zh):
                nc.tensor.matmul(pb[:r, :], zt[:, g0:g0 + r],
                                 ws_all[m][nzx + i], start=False, stop=False)

        def zx_side(ch, m, g0, r):
            pb = pbs[ch]
            zzx = [] if "nomm" in ABLATE else z_zx
            for i, zt in enumerate(zzx):
                nc.tensor.matmul(pb[:r, :], zt[:, g0:g0 + r], ws_all[m][i],
                                 start=False, stop=(i == nzx - 1))
            if nzx == 0 and "nomm" not in ABLATE:
                nc.tensor.matmul(pb[:r, :], z_h[-1][:, g0:g0 + r],
                                 ws_all[m][Kc - 1], start=False, stop=True)
            _evac_relu(ch, xsc[:r, ch, :], pb[:r, :])

        for ch, (m, g0, r) in enumerate(chunks[j][:nA]):
            bias_h(ch, m, g0, r)
        for ch, (m, g0, r) in enumerate(chunks[j][:nA]):
            zx_side(ch, m, g0, r)
        for ch, (m, g0, r) in enumerate(chunks[j]):
            if ch < nA:
                continue
            bias_h(ch, m, g0, r)
            zx_side(ch, m, g0, r)
        return xsc

    def transition(t_i, xsc):
        """Scatter chunk tokens into next stage's order (DRAM, token-major),
        load back contiguous 128-token chunks, PE-transpose to feature-major."""
        xb = d["xb"][t_i]
        for ch, (m, g0, r) in enumerate(chunks[t_i]):
            if "noscatter" in ABLATE:
                continue
            col = tboff[t_i] + ch
            nc.gpsimd.indirect_dma_start(
                xb.ap(),
                bass.IndirectOffsetOnAxis(ap=tbl[:r, col:col + 1], axis=0),
                xsc[:r, ch, :], None)
        bpn = bps[t_i + 1]
        ntc = (bpn + P - 1) // P                    # token chunks to load
        zb = permp.tile([P, ntc, H], BF16, tag="zb", name="zb", bufs=2)
        for c in range(ntc):
            rr = min(P, bpn - c * P)
            nc.sync.dma_start(zb[:rr, c, :], xb.ap()[c * P:c * P + rr, :])
        zx = permp.tile([P, 4, bpn], BF16, tag="zx", name="zx")
        for c in range(ntc):
            rr = min(P, bpn - c * P)
            for fk in range(4):
                pt = psp.tile([P, P], BF16, tag="pt", name="pt")
                nc.tensor.transpose(pt[:, :rr], zb[:rr, c, bass.ts(fk, P)],
                                    ident[:rr, :rr])
                if (c * 4 + fk) % 2:
                    nc.scalar.activation(zx[:, fk, c * P:c * P + rr],
                                         pt[:, :rr], COPY)
                else:
                    nc.vector.tensor_copy(zx[:, fk, c * P:c * P + rr],
                                          pt[:, :rr])
        return [zx[:, k, :] for k in range(4)]

    def grouped_big(j, z_zx, z_h, tag):
        """relu(W_j1[expert].T @ z + b): feature-major grouped output."""
        Kc = KBIG[j]
        nzx = len(z_zx)
        zs = list(z_zx) + list(z_h)
        ks = list(range(nzx, Kc)) + list(range(nzx))   # h-first
        outs = [hp.tile([P, bps[j]], BF16, tag=f"{tag}{hh}", name=f"{tag}{hh}")
                for hh in range(4)]
        for m in range(M):
            o, c = offs[j][m], caps[j][m]
            ws = load_w(j, m, Kc)
            ps = [psp.tile([P, capmax], F32, tag="pt", name="pt")
                  for hh in range(4)]
            for i, k in enumerate(ks if "nomm" not in ABLATE else ks[:1]):
                for hh in range(4):
                    nc.tensor.matmul(
                        ps[hh][:, :c], ws[k][:, bass.ts(hh, P)],
                        zs[k][:, o:o + c],
                        start=(i == 0), stop=(i == Kc - 1))
            for hh in range(4):
                nc.scalar.activation(outs[hh][:, o:o + c], ps[hh][:, :c],
                                     RELU, bias=bias_ap(2 * j + 1, hh, m))
        return outs

    # ---------------- network ----------------
    x = first_layer(0, "h")
    xsc = swapped_big(0, [], x, 0)
    zx = transition(0, xsc)
    h1 = first_layer(1, "g")
    xsc = swapped_big(1, zx, h1, 6)
    zx = transition(1, xsc)
    h2 = first_layer(2, "h")
    xsc = swapped_big(2, zx, h2, 6)
    zx = transition(2, xsc)
    h3 = first_layer(3, "g")
    x4 = grouped_big(3, zx, h3, "x4")

    # head
    ps = [psp.tile([OUT, capmax], F32, tag="pt", name="pt") for m in range(M)]
    for k in range(4):
        for m in range(M):
            nc.tensor.matmul(ps[m][:, :caps[3][m]], w32_t[m][:, k, :],
                             x4[k][:, offs[3][m]:offs[3][m] + caps[3][m]],
                             start=(k == 0), stop=(k == 3))
    out_t = outp.tile([OUT, bps[3]], F32, tag="outt", name="outt")
    for m in range(M):
        o, c = offs[3][m], caps[3][m]
        nc.scalar.activation(out_t[:, o:o + c], ps[m][:, :c], COPY)
        nc.vector.tensor_scalar_add(out_t[:, o:o + c],
                                    out_t[:, o:o + c], bh[:, m:m + 1])
    nc.sync.dma_start(d["out"].ap(), out_t[:])


def build_program(caps, reps: int = 1):
    offs, bps = zip(*[_offs(c) for c in caps])
    nch = [len(_chunks(c)) for c in caps]
    xbrows = max(bps) + P
    nc = bacc.Bacc("TRN2", target_bir_lowering=False, debug=False,
                   enable_asserts=False)
    d = {}
    for j in range(4):
        d[f"xe{j}"] = nc.dram_tensor(f"xe{j}", [P, bps[j]], BF16,
                                     kind="ExternalInput")
    d["Wf"] = nc.dram_tensor("Wf", [4, P, H], BF16, kind="ExternalInput")
    d["W01"] = nc.dram_tensor("W01", [M, H, H], BF16, kind="ExternalInput")
    d["W11"] = nc.dram_tensor("W11", [M, 2 * H, H], BF16, kind="ExternalInput")
    d["W21"] = nc.dram_tensor("W21", [M, 2 * H, H], BF16, kind="ExternalInput")
    d["W31"] = nc.dram_tensor("W31", [M, 2 * H, H], BF16, kind="ExternalInput")
    d["W32"] = nc.dram_tensor("W32", [M, H, OUT], BF16, kind="ExternalInput")
    d["bias"] = nc.dram_tensor("bias", [P, 8 * 16], F32, kind="ExternalInput")
    d["bh"] = nc.dram_tensor("bh", [OUT, 4], F32, kind="ExternalInput")
    d["brow"] = nc.dram_tensor("brow", [1, 12, H], BF16,
                               kind="ExternalInput")
    d["ones"] = nc.dram_tensor("ones", [1, P], BF16,
                               kind="ExternalInput")
    d["tbl"] = nc.dram_tensor("tbl", [P, sum(nch[:3])], I32,
                              kind="ExternalInput")
    d["ident"] = nc.dram_tensor("ident", [P, P], BF16, kind="ExternalInput")
    d["out"] = nc.dram_tensor("out", [OUT, bps[3]], F32, kind="ExternalOutput")
    d["xb"] = [nc.dram_tensor(f"xb{i}", [xbrows, H], BF16, kind="Internal")
               for i in range(3)]

    with tile.TileContext(nc) as tc, ExitStack() as ctx:
        if reps == 1:
            _emit(nc, tc, ctx, d, caps)
        else:
            with tc.For_i(0, reps, 1):
                _emit(nc, tc, ctx, d, caps)
    nc.compile()
    return nc


def _balance(idx):
    """Assign rows to cores: per-core totals == BC, per-(core,type,expert)
    counts <= cap[j][m] ~= ceil(global/8).  Greedy + swap repair; caps bump
    if repair stalls.  Returns (rows_per_core, caps)."""
    rng = np.random.RandomState(12345)
    Bn = idx.shape[1]
    cap = np.array([[int(np.ceil(np.count_nonzero(idx[j] == m) / NCORES))
                     for m in range(M)] for j in range(4)])
    cnt = np.zeros((NCORES, 4, M), np.int64)
    tot = np.zeros(NCORES, np.int64)
    assign = np.full(Bn, -1, np.int64)
    order = rng.permutation(Bn)
    for r in order:
        lbl = idx[:, r]
        best, bpen = -1, None
        for c in range(NCORES):
            if tot[c] >= BC:
                continue
            over = sum(max(0, cnt[c, j, lbl[j]] + 1 - cap[j, lbl[j]])
                       for j in range(4))
            load = sum(cnt[c, j, lbl[j]] / cap[j, lbl[j]] for j in range(4))
            pen = over * 1000 + load
            if bpen is None or pen < bpen:
                best, bpen = c, pen
        assign[r] = best
        tot[best] += 1
        for j in range(4):
            cnt[best, j, idx[j, r]] += 1

    rows_by_core = [np.nonzero(assign == c)[0] for c in range(NCORES)]
    # swap repair
    for _ in range(200):
        viol = [(c, j, m) for c in range(NCORES) for j in range(4)
                for m in range(M) if cnt[c, j, m] > cap[j, m]]
        if not viol:
            break
        fixed = False
        for c, j, m in viol:
            cand = [r for r in rows_by_core[c] if idx[j, r] == m]
            done = False
            for r in cand:
                lbl = idx[:, r]
                for c2 in rng.permutation(NCORES):
                    if c2 == c:
                        continue
                    if any(cnt[c2, jj, lbl[jj]] + 1 > cap[jj, lbl[jj]]
                           for jj in range(4)):
                        continue
                    # need a row from c2 movable to c without new violations
                    for r2 in rows_by_core[c2]:
                        l2 = idx[:, r2]
                        if l2[j] == m:
                            continue
                        ok = True
                        for jj in range(4):
                            d_ = cnt[c, jj, l2[jj]] + (1 if l2[jj] != lbl[jj]
                                                       else 0)
                            if l2[jj] != lbl[jj] and d_ > cap[jj, l2[jj]]:
                                ok = False
                                break
                        if not ok:
                            continue
                        for jj in range(4):
                            cnt[c, jj, lbl[jj]] -= 1
                            cnt[c2, jj, lbl[jj]] += 1
                            cnt[c2, jj, l2[jj]] -= 1
                            cnt[c, jj, l2[jj]] += 1
                        assign[r], assign[r2] = c2, c
                        rows_by_core[c] = np.nonzero(assign == c)[0]
                        rows_by_core[c2] = np.nonzero(assign == c2)[0]
                        done = True
                        break
                    if done:
                        break
                if done:
                    fixed = True
                    break
            if done:
                break
        if not fixed:      # stuck: relax the tightest violated cap
            c, j, m = viol[0]
            cap[j, m] += 1
    caps = []
    for j in range(4):
        row = []
        for m in range(M):
            mx = max(int(np.count_nonzero(idx[j, rows_by_core[c]] == m))
                     for c in range(NCORES))
            row.append(((mx + 7) // 8) * 8)
        caps.append(tuple(row))
    return rows_by_core, tuple(caps)


def prep_inputs(inputs):
    iv = np.asarray(inputs["input_val"], dtype=np.float32)
    feats = iv[:, :4 * FEAT]
    oh = iv[:, 4 * FEAT:4 * FEAT + 16]
    idx = np.stack([np.argmax(oh[:, 4 * j:4 * j + 4], axis=1)
                    for j in range(4)])
    rows_by_core, caps = _balance(idx)
    offs, bps = zip(*[_offs(c) for c in caps])
    chunks = [_chunks(c) for c in caps]
    nch = [len(ch) for ch in chunks]
    tboff, _ = _offs(nch[:3])
    xbrows = max(bps) + P

    tobf = lambda a: np.ascontiguousarray(
        np.asarray(a, np.float32).astype(ml_dtypes.bfloat16))

    bias = np.zeros((P, 8 * 16), np.float32)
    for j in range(4):
        bl = np.asarray(inputs[f"b{j}_0"], np.float32)
        for hh in range(4):
            for m in range(M):
                bias[:, 2 * j * 16 + hh * 4 + m] = bl[m, hh * P:(hh + 1) * P]
    b31 = np.asarray(inputs["b3_1"], np.float32)
    for hh in range(4):
        for m in range(M):
            bias[:, 7 * 16 + hh * 4 + m] = b31[m, hh * P:(hh + 1) * P]
    brow = np.zeros((1, 12, H), np.float32)
    for t, nm in enumerate(("b0_1", "b1_1", "b2_1")):
        bl = np.asarray(inputs[nm], np.float32)
        for m in range(M):
            brow[0, t * 4 + m] = bl[m]
    bh = np.ascontiguousarray(np.asarray(inputs["b3_2"], np.float32).T)
    ones = np.ones((1, P), np.float32)

    Wf = np.stack([np.asarray(inputs[f"W{j}_0"], np.float32).reshape(P, H)
                   for j in range(4)])
    shared = {
        "Wf": tobf(Wf), "bias": bias, "bh": bh,
        "ident": tobf(np.eye(P, dtype=np.float32)),
        "brow": tobf(brow), "ones": tobf(ones),
        "W01": tobf(inputs["W0_1"]), "W11": tobf(inputs["W1_1"]),
        "W21": tobf(inputs["W2_1"]), "W31": tobf(inputs["W3_1"]),
        "W32": tobf(inputs["W3_2"]),
    }

    in_maps, meta = [], []
    for c in range(NCORES):
        rows = rows_by_core[c]
        nrows = len(rows)
        orders, slots, padlists = [], [], []
        for j in range(4):
            ij = idx[j][rows]
            order = np.full(bps[j], -1, np.int64)
            slot = np.empty(nrows, np.int64)
            pads = []
            for m in range(M):
                rr = np.nonzero(ij == m)[0]
                o0 = offs[j][m]
                order[o0:o0 + len(rr)] = rr
                slot[rr] = o0 + np.arange(len(rr))
                pads.extend(range(o0 + len(rr), o0 + caps[j][m]))
            orders.append(order)
            slots.append(slot)
            padlists.append(np.array(pads, np.int64))

        xef = []
        for j in range(4):
            ij = idx[j][rows]
            fj = feats[rows][:, FEAT * j:FEAT * (j + 1)]
            xe = np.zeros((P, bps[j]), np.float32)
            for m in range(M):
                rr = np.nonzero(ij == m)[0]
                o0 = offs[j][m]
                xe[m * FEAT:(m + 1) * FEAT, o0:o0 + len(rr)] = fj[rr].T
            xef.append(xe.astype(ml_dtypes.bfloat16))

        tblv = np.full((P, sum(nch[:3])), xbrows - 1, np.int32)
        for t in range(3):
            jn = t + 1
            padnext = list(padlists[jn])
            spill = bps[jn]          # parking rows beyond Bp_next
            pi = 0
            for ch, (m, g0, r) in enumerate(chunks[t]):
                for p in range(r):
                    s = orders[t][g0 + p]
                    if s >= 0:
                        tblv[p, tboff[t] + ch] = slots[jn][s]
                    elif pi < len(padnext):
                        tblv[p, tboff[t] + ch] = padnext[pi]
                        pi += 1
                    else:
                        tblv[p, tboff[t] + ch] = spill
                        spill += 1
        in_maps.append({"xe0": xef[0], "xe1": xef[1], "xe2": xef[2],
                        "xe3": xef[3], "tbl": tblv, **shared})
        meta.append((rows, slots[3]))
    return caps, in_maps, meta


_CACHE = {}


def kernel(**inputs):
    caps, in_maps, meta = prep_inputs(inputs)
    if ("nc", caps) not in _CACHE:
        _CACHE[("nc", caps)] = build_program(caps)
    nc = _CACHE[("nc", caps)]
    res = bass_utils.run_bass_kernel_spmd(
        nc, in_maps, core_ids=list(range(NCORES)))
    out = np.empty((B, OUT), np.float32)
    for c in range(NCORES):
        o = res.results[c]["out"]
        rows, slot3 = meta[c]
        out[rows] = o[:, slot3].T
    return out


if __name__ == "__main__":
    import sys, jax
    import reference
    cpu = jax.local_devices(backend="cpu")[0]
    with jax.default_device(cpu):
        inputs = {k: np.asarray(v) for k, v in reference.setup_inputs().items()}
        exp = np.asarray(reference.reference(**inputs))
    if len(sys.argv) > 1 and sys.argv[1] == "sim":
        from concourse.bass_interp import CoreSim
        caps, in_maps, meta = prep_inputs(inputs)
        print("caps:", caps)
        nc = build_program(caps)
        sim = CoreSim(nc, trace=len(sys.argv) > 2)
        for k, v in in_maps[0].items():
            sim.tensor(k)[:] = v
        sim.simulate()
        print("sim time:", sim.time)
        o = np.asarray(sim.tensor("out"))
        rows, slot3 = meta[0]
        got0 = o[:, slot3].T
        exp0 = exp[rows]
        err = np.abs(got0 - exp0)
        print(f"sim core0 max abs err: {err.max():.3e}  "
              f"rel: {err.max()/np.abs(exp0).max():.3e}")

    else:
        got = kernel(**inputs)
        err = np.abs(got - exp)
        print(f"max abs err: {err.max():.3e}   "
              f"rel: {err.max()/np.abs(exp).max():.3e}")


# revision 21
# speedup vs baseline: 1.1373x; 1.1373x over previous
"""V4: routed kernel, all-bf16, ragged per-expert capacities, balanced cores.

Data-parallel over 8 cores, weights replicated (bf16).  Rows are assigned to
cores by a balancing pass so each (core, module-type, expert) group fits a
capacity close to ceil(global_count/8); capacities are per-(type, expert)
program constants (ragged), cutting padding vs a uniform worst-case C.

- All matmul operands are bf16 (fp32 PSUM accumulate): first layers use the
  order-agnostic expanded-input trick (K=128), big layers run routed
  (4x fewer FLOPs than dense).
- L0_1/L1_1/L2_1 run operand-swapped (lhsT = activation column block, rhs =
  that group's expert weights), PSUM output batch-major; ReLU evacuation
  writes a chunk-major token tile which a native indirect-scatter DMA writes
  to DRAM rows in the NEXT stage's expert order, and XBAR transpose-DMAs
  load it back feature-major.
- L3_1 and the head run feature-major grouped (no transition after them).
- Biases: ACT bias operand (feature-major layers, exact fp32) or a K=1
  ones-outer-product matmul into PSUM (batch-major layers, bf16 bias).
- Per-expert weight K-stacks load as ONE strided DMA per (layer, expert).
"""

import numpy as np
import ml_dtypes
from contextlib import ExitStack

import concourse.bass as bass
import concourse.bacc as bacc
import concourse.tile as tile
import concourse.mybir as mybir
from concourse import bass_utils

F32 = mybir.dt.float32
BF16 = mybir.dt.bfloat16
I32 = mybir.dt.int32
RELU = mybir.ActivationFunctionType.Relu
COPY = mybir.ActivationFunctionType.Copy

B = 8192
NCORES = 8
BC = B // NCORES
FEAT = 32
M = 4
H = 512
OUT = 8
P = 128
KBIG = [4, 8, 8, 8]
ABLATE = set()


def _chunks(caps):
    """Chunk split of ragged groups: [(m, g0, r)] with r<=128."""
    out = []
    off0 = 0
    for m in range(M):
        off = 0
        while off < caps[m]:
            r = min(P, caps[m] - off)
            out.append((m, off0 + off, r))
            off += r
        off0 += caps[m]
    return out


def _offs(caps):
    o, s = [], 0
    for c in caps:
        o.append(s)
        s += c
    return o, s


def _emit(nc, tc, ctx, d, caps):
    capmax = max(max(c) for c in caps)
    offs, bps = zip(*[_offs(c) for c in caps])        # per-type offsets, Bp
    chunks = [_chunks(c) for c in caps]               # per-type chunk lists
    nch = [len(ch) for ch in chunks]
    tboff, _ = _offs(nch[:3])

    consts = ctx.enter_context(tc.tile_pool(name="consts", bufs=1))
    wpool = ctx.enter_context(tc.tile_pool(name="wbig", bufs=4))
    hp = ctx.enter_context(tc.tile_pool(name="hacts", bufs=1))
    permp = ctx.enter_context(tc.tile_pool(name="perm", bufs=1))
    outp = ctx.enter_context(tc.tile_pool(name="outs", bufs=1))
    psp = ctx.enter_context(tc.tile_pool(name="psum", bufs=8, space="PSUM"))

    # ---------------- constants ----------------
    xe_t = []
    for j in range(4):
        t = consts.tile([P, bps[j]], BF16, tag=f"xe{j}", name=f"xe{j}")
        nc.sync.dma_start(t[:], d[f"xe{j}"].ap())
        xe_t.append(t)
    wf_t = []
    for j in range(4):
        t = consts.tile([P, H], BF16, tag=f"wf{j}", name=f"wf{j}")
        nc.sync.dma_start(t[:], d["Wf"].ap()[j, :, :])
        wf_t.append(t)
    w32_t = []
    for m in range(M):
        t = consts.tile([P, 4, OUT], BF16, tag=f"w32_{m}", name=f"w32_{m}")
        nc.sync.dma_start(
            t[:], d["W32"].ap()[m, :, :].rearrange("(a p) o -> p a o", p=P))
        w32_t.append(t)
    bias_sb = consts.tile([P, 8 * 16], F32, tag="bias", name="bias")
    nc.sync.dma_start(bias_sb[:], d["bias"].ap())
    bh = consts.tile([OUT, 4], F32, tag="bh", name="bh")
    nc.sync.dma_start(bh[:], d["bh"].ap())
    brow = consts.tile([1, 12, H], BF16, tag="brow", name="brow")
    nc.gpsimd.dma_start(brow[:], d["brow"].ap())
    ones = consts.tile([1, P], BF16, tag="ones", name="ones")
    nc.sync.dma_start(ones[:], d["ones"].ap())


    def bias_ap(layer, hh, m):
        col = layer * 16 + hh * 4 + m
        return bias_sb[:, col:col + 1]

    # ---------------- layers ----------------
    def first_layer(j, tag):
        """relu(Wf[j].T @ xe_g[j] + b_j0): 4x [128, Bp_j] bf16, feat-major."""
        outs = []
        for hpair in range(2):
            ps = [[psp.tile([P, capmax], F32, tag="pt", name="pt")
                   for m in range(M)] for _ in range(2)]
            for hi in range(2):
                hh = hpair * 2 + hi
                for m in range(M):
                    o, c = offs[j][m], caps[j][m]
                    nc.tensor.matmul(
                        ps[hi][m][:, :c], wf_t[j][:, bass.ts(hh, P)],
                        xe_t[j][:, o:o + c], start=True, stop=True)
            for hi in range(2):
                hh = hpair * 2 + hi
                t = hp.tile([P, bps[j]], BF16, tag=f"{tag}{hh}",
                            name=f"{tag}{hh}")
                for m in range(M):
                    o, c = offs[j][m], caps[j][m]
                    nc.scalar.activation(t[:, o:o + c], ps[hi][m][:, :c],
                                         RELU, bias=bias_ap(2 * j, hh, m))
                outs.append(t)
        return outs

    def load_w(j, m, Kc):
        w = wpool.tile([P, Kc, H], BF16, tag="wt", name="wt")
        if "noweights" in ABLATE:
            return [w[:, k, :] for k in range(Kc)]
        nc.sync.dma_start(
            w[:], d[f"W{j}1"].ap()[m, :, :].rearrange("(k p) h -> p k h", p=P))
        return [w[:, k, :] for k in range(Kc)]

    def _evac_relu(ch, dst, src):
        if "noevac" in ABLATE:
            return
        if ch % 2:
            nc.scalar.activation(dst, src, RELU)
        else:
            nc.vector.tensor_relu(dst, src)

    def swapped_big(j, z_zx, z_h, nA):
        """relu(W_j1[expert].T @ z + b), batch-major out -> xsc token tile.

        Chunks [0:nA) run two-pass: bias + first-layer-feature half first
        (independent of the inter-layer transition), zx half after -- PSUM
        holds the partials so PE has work while the transition DMAs run."""
        Kc = KBIG[j]
        nzx = len(z_zx)
        xsc = permp.tile([P, nch[j], H], BF16, tag="xsc", name="xsc", bufs=2)
        ws_all = [load_w(j, m, Kc) for m in range(M)]
        pbs = {}

        def bias_h(ch, m, g0, r):
            pb = psp.tile([P, H], F32, tag="pt", name="pt")
            pbs[ch] = pb
            lm = 4 * j + m
            nc.tensor.matmul(pb[:r, :], ones[:, :r],
                             brow[:, lm, :],
                             start=True, stop=False)
            zh = z_h if nzx else z_h[:-1]
            if "nomm" in ABLATE:
                zh = []
            for i, zt in enumerate(zh):
                nc.tensor.matmul(pb[:r, :], zt[:, g0:g0 + r],
                                 ws_all[m][nzx + i], start=False, stop=False)

        def zx_side(ch, m, g0, r):
            pb = pbs[ch]
            zzx = [] if "nomm" in ABLATE else z_zx
            for i, zt in enumerate(zzx):
                nc.tensor.matmul(pb[:r, :], zt[:, g0:g0 + r], ws_all[m][i],
                                 start=False, stop=(i == nzx - 1))
            if nzx == 0 and "nomm" not in ABLATE:
                nc.tensor.matmul(pb[:r, :], z_h[-1][:, g0:g0 + r],
                                 ws_all[m][Kc - 1], start=False, stop=True)
            _evac_relu(ch, xsc[:r, ch, :], pb[:r, :])

        for ch, (m, g0, r) in enumerate(chunks[j][:nA]):
            bias_h(ch, m, g0, r)
        for ch, (m, g0, r) in enumerate(chunks[j][:nA]):
            zx_side(ch, m, g0, r)
        for ch, (m, g0, r) in enumerate(chunks[j]):
            if ch < nA:
                continue
            bias_h(ch, m, g0, r)
            zx_side(ch, m, g0, r)
        return xsc

    def transition(t_i, xsc):
        """Permute+transpose to next stage order fused as PE matmuls
        against one-hot P: zx[f, new] = sum_oc xsc[:, oc, f].T @ P[:, oc, new].
        No DRAM round trip; pad slots get all-zero P columns."""
        bpn = bps[t_i + 1]
        pt_t = permp.tile([P, nch[t_i], bpn], BF16, tag="ptab", name="ptab",
                          bufs=2)
        half = (nch[t_i] + 1) // 2
        nc.sync.dma_start(pt_t[:, :half, :], d[f"pm{t_i}"].ap()[:, :half, :])
        nc.sync.dma_start(pt_t[:, half:, :], d[f"pm{t_i}"].ap()[:, half:, :])
        zx = permp.tile([P, 4, bpn], BF16, tag="zx", name="zx")
        pieces = [(0, 512), (512, 512), (1024, bpn - 1024)]
        pieces = [(o, w) for o, w in pieces if w > 0]
        nchunks = chunks[t_i]
        for po, pw in pieces:
            for fk in range(4):
                pp = psp.tile([P, 512], F32, tag="pt", name="pt")
                for oc, (m, g0, r) in enumerate(nchunks):
                    nc.tensor.matmul(pp[:, :pw],
                                     xsc[:r, oc, bass.ts(fk, P)],
                                     pt_t[:r, oc, po:po + pw],
                                     start=(oc == 0),
                                     stop=(oc == len(nchunks) - 1))
                if fk % 2:
                    nc.scalar.activation(zx[:, fk, po:po + pw], pp[:, :pw],
                                         COPY)
                else:
                    nc.vector.tensor_copy(zx[:, fk, po:po + pw], pp[:, :pw])
        return [zx[:, k, :] for k in range(4)]

    def grouped_big(j, z_zx, z_h, tag):
        """relu(W_j1[expert].T @ z + b): feature-major grouped output."""
        Kc = KBIG[j]
        nzx = len(z_zx)
        zs = list(z_zx) + list(z_h)
        ks = list(range(nzx, Kc)) + list(range(nzx))   # h-first
        outs = [hp.tile([P, bps[j]], BF16, tag=f"{tag}{hh}", name=f"{tag}{hh}")
                for hh in range(4)]
        for m in range(M):
            o, c = offs[j][m], caps[j][m]
            ws = load_w(j, m, Kc)
            ps = [psp.tile([P, capmax], F32, tag="pt", name="pt")
                  for hh in range(4)]
            for i, k in enumerate(ks if "nomm" not in ABLATE else ks[:1]):
                for hh in range(4):
                    nc.tensor.matmul(
                        ps[hh][:, :c], ws[k][:, bass.ts(hh, P)],
                        zs[k][:, o:o + c],
                        start=(i == 0), stop=(i == Kc - 1))
            for hh in range(4):
                nc.scalar.activation(outs[hh][:, o:o + c], ps[hh][:, :c],
                                     RELU, bias=bias_ap(2 * j + 1, hh, m))
        return outs

    # ---------------- network ----------------
    x = first_layer(0, "h")
    xsc = swapped_big(0, [], x, 0)
    zx = transition(0, xsc)
    h1 = first_layer(1, "g")
    xsc = swapped_big(1, zx, h1, 6)
    zx = transition(1, xsc)
    h2 = first_layer(2, "h")
    xsc = swapped_big(2, zx, h2, 6)
    zx = transition(2, xsc)
    h3 = first_layer(3, "g")
    x4 = grouped_big(3, zx, h3, "x4")

    # head
    ps = [psp.tile([OUT, capmax], F32, tag="pt", name="pt") for m in range(M)]
    for k in range(4):
        for m in range(M):
            nc.tensor.matmul(ps[m][:, :caps[3][m]], w32_t[m][:, k, :],
                             x4[k][:, offs[3][m]:offs[3][m] + caps[3][m]],
                             start=(k == 0), stop=(k == 3))
    out_t = outp.tile([OUT, bps[3]], F32, tag="outt", name="outt")
    for m in range(M):
        o, c = offs[3][m], caps[3][m]
        nc.scalar.activation(out_t[:, o:o + c], ps[m][:, :c], COPY)
        nc.vector.tensor_scalar_add(out_t[:, o:o + c],
                                    out_t[:, o:o + c], bh[:, m:m + 1])
    nc.sync.dma_start(d["out"].ap(), out_t[:])


def build_program(caps, reps: int = 1):
    offs, bps = zip(*[_offs(c) for c in caps])
    nch = [len(_chunks(c)) for c in caps]
    chunks_all = [_chunks(c) for c in caps]
    nc = bacc.Bacc("TRN2", target_bir_lowering=False, debug=False,
                   enable_asserts=False)
    d = {}
    for j in range(4):
        d[f"xe{j}"] = nc.dram_tensor(f"xe{j}", [P, bps[j]], BF16,
                                     kind="ExternalInput")
    d["Wf"] = nc.dram_tensor("Wf", [4, P, H], BF16, kind="ExternalInput")
    d["W01"] = nc.dram_tensor("W01", [M, H, H], BF16, kind="ExternalInput")
    d["W11"] = nc.dram_tensor("W11", [M, 2 * H, H], BF16, kind="ExternalInput")
    d["W21"] = nc.dram_tensor("W21", [M, 2 * H, H], BF16, kind="ExternalInput")
    d["W31"] = nc.dram_tensor("W31", [M, 2 * H, H], BF16, kind="ExternalInput")
    d["W32"] = nc.dram_tensor("W32", [M, H, OUT], BF16, kind="ExternalInput")
    d["bias"] = nc.dram_tensor("bias", [P, 8 * 16], F32, kind="ExternalInput")
    d["bh"] = nc.dram_tensor("bh", [OUT, 4], F32, kind="ExternalInput")
    d["brow"] = nc.dram_tensor("brow", [1, 12, H], BF16,
                               kind="ExternalInput")
    d["ones"] = nc.dram_tensor("ones", [1, P], BF16,
                               kind="ExternalInput")
    for t in range(3):
        d[f"pm{t}"] = nc.dram_tensor(f"pm{t}", [P, nch[t], bps[t + 1]], BF16,
                                     kind="ExternalInput")
    d["out"] = nc.dram_tensor("out", [OUT, bps[3]], F32, kind="ExternalOutput")

    with tile.TileContext(nc) as tc, ExitStack() as ctx:
        if reps == 1:
            _emit(nc, tc, ctx, d, caps)
        else:
            with tc.For_i(0, reps, 1):
                _emit(nc, tc, ctx, d, caps)
    nc.compile()
    return nc


def _balance(idx):
    """Assign rows to cores: per-core totals == BC, per-(core,type,expert)
    counts <= cap[j][m] ~= ceil(global/8).  Greedy + swap repair; caps bump
    if repair stalls.  Returns (rows_per_core, caps)."""
    rng = np.random.RandomState(12345)
    Bn = idx.shape[1]
    cap = np.array([[int(np.ceil(np.count_nonzero(idx[j] == m) / NCORES))
                     for m in range(M)] for j in range(4)])
    cnt = np.zeros((NCORES, 4, M), np.int64)
    tot = np.zeros(NCORES, np.int64)
    assign = np.full(Bn, -1, np.int64)
    order = rng.permutation(Bn)
    for r in order:
        lbl = idx[:, r]
        best, bpen = -1, None
        for c in range(NCORES):
            if tot[c] >= BC:
                continue
            over = sum(max(0, cnt[c, j, lbl[j]] + 1 - cap[j, lbl[j]])
                       for j in range(4))
            load = sum(cnt[c, j, lbl[j]] / cap[j, lbl[j]] for j in range(4))
            pen = over * 1000 + load
            if bpen is None or pen < bpen:
                best, bpen = c, pen
        assign[r] = best
        tot[best] += 1
        for j in range(4):
            cnt[best, j, idx[j, r]] += 1

    rows_by_core = [np.nonzero(assign == c)[0] for c in range(NCORES)]
    # swap repair
    for _ in range(200):
        viol = [(c, j, m) for c in range(NCORES) for j in range(4)
                for m in range(M) if cnt[c, j, m] > cap[j, m]]
        if not viol:
            break
        fixed = False
        for c, j, m in viol:
            cand = [r for r in rows_by_core[c] if idx[j, r] == m]
            done = False
            for r in cand:
                lbl = idx[:, r]
                for c2 in rng.permutation(NCORES):
                    if c2 == c:
                        continue
                    if any(cnt[c2, jj, lbl[jj]] + 1 > cap[jj, lbl[jj]]
                           for jj in range(4)):
                        continue
                    # need a row from c2 movable to c without new violations
                    for r2 in rows_by_core[c2]:
                        l2 = idx[:, r2]
                        if l2[j] == m:
                            continue
                        ok = True
                        for jj in range(4):
                            d_ = cnt[c, jj, l2[jj]] + (1 if l2[jj] != lbl[jj]
                                                       else 0)
                            if l2[jj] != lbl[jj] and d_ > cap[jj, l2[jj]]:
                                ok = False
                                break
                        if not ok:
                            continue
                        for jj in range(4):
                            cnt[c, jj, lbl[jj]] -= 1
                            cnt[c2, jj, lbl[jj]] += 1
                            cnt[c2, jj, l2[jj]] -= 1
                            cnt[c, jj, l2[jj]] += 1
                        assign[r], assign[r2] = c2, c
                        rows_by_core[c] = np.nonzero(assign == c)[0]
                        rows_by_core[c2] = np.nonzero(assign == c2)[0]
                        done = True
                        break
                    if done:
                        break
                if done:
                    fixed = True
                    break
            if done:
                break
        if not fixed:      # stuck: relax the tightest violated cap
            c, j, m = viol[0]
            cap[j, m] += 1
    caps = []
    for j in range(4):
        row = []
        for m in range(M):
            mx = max(int(np.count_nonzero(idx[j, rows_by_core[c]] == m))
                     for c in range(NCORES))
            row.append(((mx + 7) // 8) * 8)
        caps.append(tuple(row))
    return rows_by_core, tuple(caps)


def prep_inputs(inputs):
    iv = np.asarray(inputs["input_val"], dtype=np.float32)
    feats = iv[:, :4 * FEAT]
    oh = iv[:, 4 * FEAT:4 * FEAT + 16]
    idx = np.stack([np.argmax(oh[:, 4 * j:4 * j + 4], axis=1)
                    for j in range(4)])
    rows_by_core, caps = _balance(idx)
    offs, bps = zip(*[_offs(c) for c in caps])
    chunks = [_chunks(c) for c in caps]
    nch = [len(ch) for ch in chunks]

    tobf = lambda a: np.ascontiguousarray(
        np.asarray(a, np.float32).astype(ml_dtypes.bfloat16))

    bias = np.zeros((P, 8 * 16), np.float32)
    for j in range(4):
        bl = np.asarray(inputs[f"b{j}_0"], np.float32)
        for hh in range(4):
            for m in range(M):
                bias[:, 2 * j * 16 + hh * 4 + m] = bl[m, hh * P:(hh + 1) * P]
    b31 = np.asarray(inputs["b3_1"], np.float32)
    for hh in range(4):
        for m in range(M):
            bias[:, 7 * 16 + hh * 4 + m] = b31[m, hh * P:(hh + 1) * P]
    brow = np.zeros((1, 12, H), np.float32)
    for t, nm in enumerate(("b0_1", "b1_1", "b2_1")):
        bl = np.asarray(inputs[nm], np.float32)
        for m in range(M):
            brow[0, t * 4 + m] = bl[m]
    bh = np.ascontiguousarray(np.asarray(inputs["b3_2"], np.float32).T)
    ones = np.ones((1, P), np.float32)

    Wf = np.stack([np.asarray(inputs[f"W{j}_0"], np.float32).reshape(P, H)
                   for j in range(4)])
    shared = {
        "Wf": tobf(Wf), "bias": bias, "bh": bh,
        "brow": tobf(brow), "ones": tobf(ones),
        "W01": tobf(inputs["W0_1"]), "W11": tobf(inputs["W1_1"]),
        "W21": tobf(inputs["W2_1"]), "W31": tobf(inputs["W3_1"]),
        "W32": tobf(inputs["W3_2"]),
    }

    in_maps, meta = [], []
    for c in range(NCORES):
        rows = rows_by_core[c]
        nrows = len(rows)
        orders, slots, padlists = [], [], []
        for j in range(4):
            ij = idx[j][rows]
            order = np.full(bps[j], -1, np.int64)
            slot = np.empty(nrows, np.int64)
            pads = []
            for m in range(M):
                rr = np.nonzero(ij == m)[0]
                o0 = offs[j][m]
                order[o0:o0 + len(rr)] = rr
                slot[rr] = o0 + np.arange(len(rr))
                pads.extend(range(o0 + len(rr), o0 + caps[j][m]))
            orders.append(order)
            slots.append(slot)
            padlists.append(np.array(pads, np.int64))

        xef = []
        for j in range(4):
            ij = idx[j][rows]
            fj = feats[rows][:, FEAT * j:FEAT * (j + 1)]
            xe = np.zeros((P, bps[j]), np.float32)
            for m in range(M):
                rr = np.nonzero(ij == m)[0]
                o0 = offs[j][m]
                xe[m * FEAT:(m + 1) * FEAT, o0:o0 + len(rr)] = fj[rr].T
            xef.append(xe.astype(ml_dtypes.bfloat16))

        pms = {}
        for t in range(3):
            bpn = bps[t + 1]
            pm = np.zeros((P, nch[t], bpn), np.float32)
            for oc, (m, g0, r) in enumerate(chunks[t]):
                for p in range(r):
                    sr = orders[t][g0 + p]
                    if sr >= 0:
                        pm[p, oc, slots[t + 1][sr]] = 1.0
            pms[f"pm{t}"] = pm.astype(ml_dtypes.bfloat16)
        in_maps.append({"xe0": xef[0], "xe1": xef[1], "xe2": xef[2],
                        "xe3": xef[3], **pms, **shared})
        meta.append((rows, slots[3]))
    return caps, in_maps, meta


_CACHE = {}


def kernel(**inputs):
    caps, in_maps, meta = prep_inputs(inputs)
    if ("nc", caps) not in _CACHE:
        _CACHE[("nc", caps)] = build_program(caps)
    nc = _CACHE[("nc", caps)]
    res = bass_utils.run_bass_kernel_spmd(
        nc, in_maps, core_ids=list(range(NCORES)))
    out = np.empty((B, OUT), np.float32)
    for c in range(NCORES):
        o = res.results[c]["out"]
        rows, slot3 = meta[c]
        out[rows] = o[:, slot3].T
    return out


if __name__ == "__main__":
    import sys, jax
    import reference
    cpu = jax.local_devices(backend="cpu")[0]
    with jax.default_device(cpu):
        inputs = {k: np.asarray(v) for k, v in reference.setup_inputs().items()}
        exp = np.asarray(reference.reference(**inputs))
    if len(sys.argv) > 1 and sys.argv[1] == "sim":
        from concourse.bass_interp import CoreSim
        caps, in_maps, meta = prep_inputs(inputs)
        print("caps:", caps)
        nc = build_program(caps)
        sim = CoreSim(nc, trace=len(sys.argv) > 2)
        for k, v in in_maps[0].items():
            sim.tensor(k)[:] = v
        sim.simulate()
        print("sim time:", sim.time)
        o = np.asarray(sim.tensor("out"))
        rows, slot3 = meta[0]
        got0 = o[:, slot3].T
        exp0 = exp[rows]
        err = np.abs(got0 - exp0)
        print(f"sim core0 max abs err: {err.max():.3e}  "
              f"rel: {err.max()/np.abs(exp0).max():.3e}")

    else:
        got = kernel(**inputs)
        err = np.abs(got - exp)
        print(f"max abs err: {err.max():.3e}   "
              f"rel: {err.max()/np.abs(exp).max():.3e}")


# revision 22
# speedup vs baseline: 1.2144x; 1.0678x over previous
"""V4: routed kernel, all-bf16, ragged per-expert capacities, balanced cores.

Data-parallel over 8 cores, weights replicated (bf16).  Rows are assigned to
cores by a balancing pass so each (core, module-type, expert) group fits a
capacity close to ceil(global_count/8); capacities are per-(type, expert)
program constants (ragged), cutting padding vs a uniform worst-case C.

- All matmul operands are bf16 (fp32 PSUM accumulate): first layers use the
  order-agnostic expanded-input trick (K=128), big layers run routed
  (4x fewer FLOPs than dense).
- L0_1/L1_1/L2_1 run operand-swapped (lhsT = activation column block, rhs =
  that group's expert weights), PSUM output batch-major; ReLU evacuation
  writes a chunk-major token tile which a native indirect-scatter DMA writes
  to DRAM rows in the NEXT stage's expert order, and XBAR transpose-DMAs
  load it back feature-major.
- L3_1 and the head run feature-major grouped (no transition after them).
- Biases: ACT bias operand (feature-major layers, exact fp32) or a K=1
  ones-outer-product matmul into PSUM (batch-major layers, bf16 bias).
- Per-expert weight K-stacks load as ONE strided DMA per (layer, expert).
"""

import numpy as np
import ml_dtypes
from contextlib import ExitStack

import concourse.bass as bass
import concourse.bacc as bacc
import concourse.tile as tile
import concourse.mybir as mybir
from concourse import bass_utils

F32 = mybir.dt.float32
BF16 = mybir.dt.bfloat16
I32 = mybir.dt.int32
RELU = mybir.ActivationFunctionType.Relu
COPY = mybir.ActivationFunctionType.Copy

B = 8192
NCORES = 8
BC = B // NCORES
FEAT = 32
M = 4
H = 512
OUT = 8
P = 128
KBIG = [4, 8, 8, 8]
ABLATE = set()


def _chunks(caps):
    """Chunk split of ragged groups: [(m, g0, r)] with r<=128."""
    out = []
    off0 = 0
    for m in range(M):
        off = 0
        while off < caps[m]:
            r = min(P, caps[m] - off)
            out.append((m, off0 + off, r))
            off += r
        off0 += caps[m]
    return out


def _offs(caps):
    o, s = [], 0
    for c in caps:
        o.append(s)
        s += c
    return o, s


def _emit(nc, tc, ctx, d, caps):
    capmax = max(max(c) for c in caps)
    offs, bps = zip(*[_offs(c) for c in caps])        # per-type offsets, Bp
    chunks = [_chunks(c) for c in caps]               # per-type chunk lists
    nch = [len(ch) for ch in chunks]
    tboff, _ = _offs(nch[:3])

    consts = ctx.enter_context(tc.tile_pool(name="consts", bufs=1))
    wpool = ctx.enter_context(tc.tile_pool(name="wbig", bufs=4))
    hp = ctx.enter_context(tc.tile_pool(name="hacts", bufs=1))
    permp = ctx.enter_context(tc.tile_pool(name="perm", bufs=1))
    outp = ctx.enter_context(tc.tile_pool(name="outs", bufs=1))
    psp = ctx.enter_context(tc.tile_pool(name="psum", bufs=8, space="PSUM"))

    # ---------------- constants ----------------
    xe_t = []
    for j in range(4):
        t = consts.tile([P, bps[j]], BF16, tag=f"xe{j}", name=f"xe{j}")
        nc.sync.dma_start(t[:], d[f"xe{j}"].ap())
        xe_t.append(t)
    wf_t = []
    for j in range(4):
        t = consts.tile([P, H], BF16, tag=f"wf{j}", name=f"wf{j}")
        nc.sync.dma_start(t[:], d["Wf"].ap()[j, :, :])
        wf_t.append(t)
    w32_t = []
    for m in range(M):
        t = consts.tile([P, 4, OUT], BF16, tag=f"w32_{m}", name=f"w32_{m}")
        nc.sync.dma_start(
            t[:], d["W32"].ap()[m, :, :].rearrange("(a p) o -> p a o", p=P))
        w32_t.append(t)
    bias_sb = consts.tile([P, 8 * 16], F32, tag="bias", name="bias")
    nc.sync.dma_start(bias_sb[:], d["bias"].ap())
    bh = consts.tile([OUT, 4], F32, tag="bh", name="bh")
    nc.sync.dma_start(bh[:], d["bh"].ap())
    brow = consts.tile([1, 12, H], BF16, tag="brow", name="brow")
    nc.gpsimd.dma_start(brow[:], d["brow"].ap())
    ones = consts.tile([1, P], BF16, tag="ones", name="ones")
    nc.sync.dma_start(ones[:], d["ones"].ap())
    iota = consts.tile([P, max(bps)], F32, tag="iota", name="iota")
    nc.sync.dma_start(iota[:], d["iota"].ap())
    tbl = consts.tile([P, sum(nch[:3])], F32, tag="tbl", name="tbl")
    nc.sync.dma_start(tbl[:], d["tbl"].ap())


    def bias_ap(layer, hh, m):
        col = layer * 16 + hh * 4 + m
        return bias_sb[:, col:col + 1]

    # ---------------- layers ----------------
    def first_layer(j, tag):
        """relu(Wf[j].T @ xe_g[j] + b_j0): 4x [128, Bp_j] bf16, feat-major."""
        outs = []
        for hpair in range(2):
            ps = [[psp.tile([P, capmax], F32, tag="pt", name="pt")
                   for m in range(M)] for _ in range(2)]
            for hi in range(2):
                hh = hpair * 2 + hi
                for m in range(M):
                    o, c = offs[j][m], caps[j][m]
                    nc.tensor.matmul(
                        ps[hi][m][:, :c], wf_t[j][:, bass.ts(hh, P)],
                        xe_t[j][:, o:o + c], start=True, stop=True)
            for hi in range(2):
                hh = hpair * 2 + hi
                t = hp.tile([P, bps[j]], BF16, tag=f"{tag}{hh}",
                            name=f"{tag}{hh}")
                for m in range(M):
                    o, c = offs[j][m], caps[j][m]
                    nc.scalar.activation(t[:, o:o + c], ps[hi][m][:, :c],
                                         RELU, bias=bias_ap(2 * j, hh, m))
                outs.append(t)
        return outs

    def load_w(j, m, Kc):
        w = wpool.tile([P, Kc, H], BF16, tag="wt", name="wt")
        if "noweights" in ABLATE:
            return [w[:, k, :] for k in range(Kc)]
        nc.sync.dma_start(
            w[:], d[f"W{j}1"].ap()[m, :, :].rearrange("(k p) h -> p k h", p=P))
        return [w[:, k, :] for k in range(Kc)]

    def _evac_relu(ch, dst, src):
        if "noevac" in ABLATE:
            return
        if ch % 2:
            nc.scalar.activation(dst, src, RELU)
        else:
            nc.vector.tensor_relu(dst, src)

    def swapped_big(j, z_zx, z_h, nA):
        """relu(W_j1[expert].T @ z + b), batch-major out -> xsc token tile.

        Chunks [0:nA) run two-pass: bias + first-layer-feature half first
        (independent of the inter-layer transition), zx half after -- PSUM
        holds the partials so PE has work while the transition DMAs run."""
        Kc = KBIG[j]
        nzx = len(z_zx)
        xsc = permp.tile([P, nch[j], H], BF16, tag="xsc", name="xsc", bufs=2)
        ws_all = [load_w(j, m, Kc) for m in range(M)]
        pbs = {}

        def bias_h(ch, m, g0, r):
            pb = psp.tile([P, H], F32, tag="pt", name="pt")
            pbs[ch] = pb
            lm = 4 * j + m
            nc.tensor.matmul(pb[:r, :], ones[:, :r],
                             brow[:, lm, :],
                             start=True, stop=False)
            zh = z_h if nzx else z_h[:-1]
            if "nomm" in ABLATE:
                zh = []
            for i, zt in enumerate(zh):
                nc.tensor.matmul(pb[:r, :], zt[:, g0:g0 + r],
                                 ws_all[m][nzx + i], start=False, stop=False)

        def zx_side(ch, m, g0, r):
            pb = pbs[ch]
            zzx = [] if "nomm" in ABLATE else z_zx
            for i, zt in enumerate(zzx):
                nc.tensor.matmul(pb[:r, :], zt[:, g0:g0 + r], ws_all[m][i],
                                 start=False, stop=(i == nzx - 1))
            if nzx == 0 and "nomm" not in ABLATE:
                nc.tensor.matmul(pb[:r, :], z_h[-1][:, g0:g0 + r],
                                 ws_all[m][Kc - 1], start=False, stop=True)
            _evac_relu(ch, xsc[:r, ch, :], pb[:r, :])

        for ch, (m, g0, r) in enumerate(chunks[j][:nA]):
            bias_h(ch, m, g0, r)
        for ch, (m, g0, r) in enumerate(chunks[j][:nA]):
            zx_side(ch, m, g0, r)
        for ch, (m, g0, r) in enumerate(chunks[j]):
            if ch < nA:
                continue
            bias_h(ch, m, g0, r)
            zx_side(ch, m, g0, r)
        return xsc

    def transition_prep(t_i):
        """Build one-hot P on DVE from the slot table: P[k,oc,n] =
        (tbl[k,oc] == n).  Pads are -1 and match nothing."""
        bpn = bps[t_i + 1]
        pt_t = permp.tile([P, nch[t_i], bpn], BF16, tag="ptab", name="ptab",
                          bufs=2)
        for oc in range(nch[t_i]):
            nc.vector.tensor_single_scalar(
                pt_t[:, oc, :], iota[:, :bpn],
                tbl[:, tboff[t_i] + oc:tboff[t_i] + oc + 1],
                mybir.AluOpType.is_equal)
        return pt_t

    def transition(t_i, xsc, pt_t):
        """Permute+transpose to next stage order fused as PE matmuls
        against one-hot P: zx[f, new] = sum_oc xsc[:, oc, f].T @ P[:, oc, new].
        No DRAM round trip; pad slots get all-zero P columns."""
        bpn = bps[t_i + 1]
        zx = permp.tile([P, 4, bpn], BF16, tag="zx", name="zx")
        pieces = [(0, 512), (512, 512), (1024, bpn - 1024)]
        pieces = [(o, w) for o, w in pieces if w > 0]
        nchunks = chunks[t_i]
        for po, pw in pieces:
            for fk in range(4):
                pp = psp.tile([P, 512], F32, tag="pt", name="pt")
                for oc, (m, g0, r) in enumerate(nchunks):
                    nc.tensor.matmul(pp[:, :pw],
                                     xsc[:r, oc, bass.ts(fk, P)],
                                     pt_t[:r, oc, po:po + pw],
                                     start=(oc == 0),
                                     stop=(oc == len(nchunks) - 1))
                if fk % 2:
                    nc.scalar.activation(zx[:, fk, po:po + pw], pp[:, :pw],
                                         COPY)
                else:
                    nc.vector.tensor_copy(zx[:, fk, po:po + pw], pp[:, :pw])
        return [zx[:, k, :] for k in range(4)]

    def grouped_big(j, z_zx, z_h, tag):
        """relu(W_j1[expert].T @ z + b): feature-major grouped output."""
        Kc = KBIG[j]
        nzx = len(z_zx)
        zs = list(z_zx) + list(z_h)
        ks = list(range(nzx, Kc)) + list(range(nzx))   # h-first
        outs = [hp.tile([P, bps[j]], BF16, tag=f"{tag}{hh}", name=f"{tag}{hh}")
                for hh in range(4)]
        for m in range(M):
            o, c = offs[j][m], caps[j][m]
            ws = load_w(j, m, Kc)
            ps = [psp.tile([P, capmax], F32, tag="pt", name="pt")
                  for hh in range(4)]
            for i, k in enumerate(ks if "nomm" not in ABLATE else ks[:1]):
                for hh in range(4):
                    nc.tensor.matmul(
                        ps[hh][:, :c], ws[k][:, bass.ts(hh, P)],
                        zs[k][:, o:o + c],
                        start=(i == 0), stop=(i == Kc - 1))
            for hh in range(4):
                nc.scalar.activation(outs[hh][:, o:o + c], ps[hh][:, :c],
                                     RELU, bias=bias_ap(2 * j + 1, hh, m))
        return outs

    # ---------------- network ----------------
    x = first_layer(0, "h")
    pt0 = transition_prep(0)
    xsc = swapped_big(0, [], x, 0)
    zx = transition(0, xsc, pt0)
    h1 = first_layer(1, "g")
    pt1 = transition_prep(1)
    xsc = swapped_big(1, zx, h1, 6)
    zx = transition(1, xsc, pt1)
    h2 = first_layer(2, "h")
    pt2 = transition_prep(2)
    xsc = swapped_big(2, zx, h2, 6)
    zx = transition(2, xsc, pt2)
    h3 = first_layer(3, "g")
    x4 = grouped_big(3, zx, h3, "x4")

    # head
    ps = [psp.tile([OUT, capmax], F32, tag="pt", name="pt") for m in range(M)]
    for k in range(4):
        for m in range(M):
            nc.tensor.matmul(ps[m][:, :caps[3][m]], w32_t[m][:, k, :],
                             x4[k][:, offs[3][m]:offs[3][m] + caps[3][m]],
                             start=(k == 0), stop=(k == 3))
    out_t = outp.tile([OUT, bps[3]], F32, tag="outt", name="outt")
    for m in range(M):
        o, c = offs[3][m], caps[3][m]
        nc.scalar.activation(out_t[:, o:o + c], ps[m][:, :c], COPY)
        nc.vector.tensor_scalar_add(out_t[:, o:o + c],
                                    out_t[:, o:o + c], bh[:, m:m + 1])
    nc.sync.dma_start(d["out"].ap(), out_t[:])


def build_program(caps, reps: int = 1):
    offs, bps = zip(*[_offs(c) for c in caps])
    nch = [len(_chunks(c)) for c in caps]
    chunks_all = [_chunks(c) for c in caps]
    nc = bacc.Bacc("TRN2", target_bir_lowering=False, debug=False,
                   enable_asserts=False)
    d = {}
    for j in range(4):
        d[f"xe{j}"] = nc.dram_tensor(f"xe{j}", [P, bps[j]], BF16,
                                     kind="ExternalInput")
    d["Wf"] = nc.dram_tensor("Wf", [4, P, H], BF16, kind="ExternalInput")
    d["W01"] = nc.dram_tensor("W01", [M, H, H], BF16, kind="ExternalInput")
    d["W11"] = nc.dram_tensor("W11", [M, 2 * H, H], BF16, kind="ExternalInput")
    d["W21"] = nc.dram_tensor("W21", [M, 2 * H, H], BF16, kind="ExternalInput")
    d["W31"] = nc.dram_tensor("W31", [M, 2 * H, H], BF16, kind="ExternalInput")
    d["W32"] = nc.dram_tensor("W32", [M, H, OUT], BF16, kind="ExternalInput")
    d["bias"] = nc.dram_tensor("bias", [P, 8 * 16], F32, kind="ExternalInput")
    d["bh"] = nc.dram_tensor("bh", [OUT, 4], F32, kind="ExternalInput")
    d["brow"] = nc.dram_tensor("brow", [1, 12, H], BF16,
                               kind="ExternalInput")
    d["ones"] = nc.dram_tensor("ones", [1, P], BF16,
                               kind="ExternalInput")
    d["iota"] = nc.dram_tensor("iota", [P, max(bps)], F32,
                               kind="ExternalInput")
    d["tbl"] = nc.dram_tensor("tbl", [P, sum(nch[:3])], F32,
                              kind="ExternalInput")
    d["out"] = nc.dram_tensor("out", [OUT, bps[3]], F32, kind="ExternalOutput")

    with tile.TileContext(nc) as tc, ExitStack() as ctx:
        if reps == 1:
            _emit(nc, tc, ctx, d, caps)
        else:
            with tc.For_i(0, reps, 1):
                _emit(nc, tc, ctx, d, caps)
    nc.compile()
    return nc


def _balance(idx):
    """Assign rows to cores: per-core totals == BC, per-(core,type,expert)
    counts <= cap[j][m] ~= ceil(global/8).  Greedy + swap repair; caps bump
    if repair stalls.  Returns (rows_per_core, caps)."""
    rng = np.random.RandomState(12345)
    Bn = idx.shape[1]
    cap = np.array([[int(np.ceil(np.count_nonzero(idx[j] == m) / NCORES))
                     for m in range(M)] for j in range(4)])
    cnt = np.zeros((NCORES, 4, M), np.int64)
    tot = np.zeros(NCORES, np.int64)
    assign = np.full(Bn, -1, np.int64)
    order = rng.permutation(Bn)
    for r in order:
        lbl = idx[:, r]
        best, bpen = -1, None
        for c in range(NCORES):
            if tot[c] >= BC:
                continue
            over = sum(max(0, cnt[c, j, lbl[j]] + 1 - cap[j, lbl[j]])
                       for j in range(4))
            load = sum(cnt[c, j, lbl[j]] / cap[j, lbl[j]] for j in range(4))
            pen = over * 1000 + load
            if bpen is None or pen < bpen:
                best, bpen = c, pen
        assign[r] = best
        tot[best] += 1
        for j in range(4):
            cnt[best, j, idx[j, r]] += 1

    rows_by_core = [np.nonzero(assign == c)[0] for c in range(NCORES)]
    # swap repair
    for _ in range(200):
        viol = [(c, j, m) for c in range(NCORES) for j in range(4)
                for m in range(M) if cnt[c, j, m] > cap[j, m]]
        if not viol:
            break
        fixed = False
        for c, j, m in viol:
            cand = [r for r in rows_by_core[c] if idx[j, r] == m]
            done = False
            for r in cand:
                lbl = idx[:, r]
                for c2 in rng.permutation(NCORES):
                    if c2 == c:
                        continue
                    if any(cnt[c2, jj, lbl[jj]] + 1 > cap[jj, lbl[jj]]
                           for jj in range(4)):
                        continue
                    # need a row from c2 movable to c without new violations
                    for r2 in rows_by_core[c2]:
                        l2 = idx[:, r2]
                        if l2[j] == m:
                            continue
                        ok = True
                        for jj in range(4):
                            d_ = cnt[c, jj, l2[jj]] + (1 if l2[jj] != lbl[jj]
                                                       else 0)
                            if l2[jj] != lbl[jj] and d_ > cap[jj, l2[jj]]:
                                ok = False
                                break
                        if not ok:
                            continue
                        for jj in range(4):
                            cnt[c, jj, lbl[jj]] -= 1
                            cnt[c2, jj, lbl[jj]] += 1
                            cnt[c2, jj, l2[jj]] -= 1
                            cnt[c, jj, l2[jj]] += 1
                        assign[r], assign[r2] = c2, c
                        rows_by_core[c] = np.nonzero(assign == c)[0]
                        rows_by_core[c2] = np.nonzero(assign == c2)[0]
                        done = True
                        break
                    if done:
                        break
                if done:
                    fixed = True
                    break
            if done:
                break
        if not fixed:      # stuck: relax the tightest violated cap
            c, j, m = viol[0]
            cap[j, m] += 1
    caps = []
    for j in range(4):
        row = []
        for m in range(M):
            mx = max(int(np.count_nonzero(idx[j, rows_by_core[c]] == m))
                     for c in range(NCORES))
            row.append(((mx + 7) // 8) * 8)
        caps.append(tuple(row))
    return rows_by_core, tuple(caps)


def prep_inputs(inputs):
    iv = np.asarray(inputs["input_val"], dtype=np.float32)
    feats = iv[:, :4 * FEAT]
    oh = iv[:, 4 * FEAT:4 * FEAT + 16]
    idx = np.stack([np.argmax(oh[:, 4 * j:4 * j + 4], axis=1)
                    for j in range(4)])
    rows_by_core, caps = _balance(idx)
    offs, bps = zip(*[_offs(c) for c in caps])
    chunks = [_chunks(c) for c in caps]
    nch = [len(ch) for ch in chunks]

    tobf = lambda a: np.ascontiguousarray(
        np.asarray(a, np.float32).astype(ml_dtypes.bfloat16))

    bias = np.zeros((P, 8 * 16), np.float32)
    for j in range(4):
        bl = np.asarray(inputs[f"b{j}_0"], np.float32)
        for hh in range(4):
            for m in range(M):
                bias[:, 2 * j * 16 + hh * 4 + m] = bl[m, hh * P:(hh + 1) * P]
    b31 = np.asarray(inputs["b3_1"], np.float32)
    for hh in range(4):
        for m in range(M):
            bias[:, 7 * 16 + hh * 4 + m] = b31[m, hh * P:(hh + 1) * P]
    brow = np.zeros((1, 12, H), np.float32)
    for t, nm in enumerate(("b0_1", "b1_1", "b2_1")):
        bl = np.asarray(inputs[nm], np.float32)
        for m in range(M):
            brow[0, t * 4 + m] = bl[m]
    bh = np.ascontiguousarray(np.asarray(inputs["b3_2"], np.float32).T)
    ones = np.ones((1, P), np.float32)

    Wf = np.stack([np.asarray(inputs[f"W{j}_0"], np.float32).reshape(P, H)
                   for j in range(4)])
    offs_l, bps_l = zip(*[_offs(c) for c in caps])
    iota_np = np.broadcast_to(np.arange(max(bps_l), dtype=np.float32),
                              (P, max(bps_l))).copy()
    shared = {
        "Wf": tobf(Wf), "bias": bias, "bh": bh, "iota": iota_np,
        "brow": tobf(brow), "ones": tobf(ones),
        "W01": tobf(inputs["W0_1"]), "W11": tobf(inputs["W1_1"]),
        "W21": tobf(inputs["W2_1"]), "W31": tobf(inputs["W3_1"]),
        "W32": tobf(inputs["W3_2"]),
    }

    in_maps, meta = [], []
    for c in range(NCORES):
        rows = rows_by_core[c]
        nrows = len(rows)
        orders, slots, padlists = [], [], []
        for j in range(4):
            ij = idx[j][rows]
            order = np.full(bps[j], -1, np.int64)
            slot = np.empty(nrows, np.int64)
            pads = []
            for m in range(M):
                rr = np.nonzero(ij == m)[0]
                o0 = offs[j][m]
                order[o0:o0 + len(rr)] = rr
                slot[rr] = o0 + np.arange(len(rr))
                pads.extend(range(o0 + len(rr), o0 + caps[j][m]))
            orders.append(order)
            slots.append(slot)
            padlists.append(np.array(pads, np.int64))

        xef = []
        for j in range(4):
            ij = idx[j][rows]
            fj = feats[rows][:, FEAT * j:FEAT * (j + 1)]
            xe = np.zeros((P, bps[j]), np.float32)
            for m in range(M):
                rr = np.nonzero(ij == m)[0]
                o0 = offs[j][m]
                xe[m * FEAT:(m + 1) * FEAT, o0:o0 + len(rr)] = fj[rr].T
            xef.append(xe.astype(ml_dtypes.bfloat16))

        tboff, _ = _offs(nch[:3])
        tblv = np.full((P, sum(nch[:3])), -1.0, np.float32)
        for t in range(3):
            for oc, (m, g0, r) in enumerate(chunks[t]):
                for p in range(r):
                    sr = orders[t][g0 + p]
                    if sr >= 0:
                        tblv[p, tboff[t] + oc] = float(slots[t + 1][sr])
        in_maps.append({"xe0": xef[0], "xe1": xef[1], "xe2": xef[2],
                        "xe3": xef[3], "tbl": tblv, **shared})
        meta.append((rows, slots[3]))
    return caps, in_maps, meta


_CACHE = {}


def kernel(**inputs):
    caps, in_maps, meta = prep_inputs(inputs)
    if ("nc", caps) not in _CACHE:
        _CACHE[("nc", caps)] = build_program(caps)
    nc = _CACHE[("nc", caps)]
    res = bass_utils.run_bass_kernel_spmd(
        nc, in_maps, core_ids=list(range(NCORES)))
    out = np.empty((B, OUT), np.float32)
    for c in range(NCORES):
        o = res.results[c]["out"]
        rows, slot3 = meta[c]
        out[rows] = o[:, slot3].T
    return out


if __name__ == "__main__":
    import sys, jax
    import reference
    cpu = jax.local_devices(backend="cpu")[0]
    with jax.default_device(cpu):
        inputs = {k: np.asarray(v) for k, v in reference.setup_inputs().items()}
        exp = np.asarray(reference.reference(**inputs))
    if len(sys.argv) > 1 and sys.argv[1] == "sim":
        from concourse.bass_interp import CoreSim
        caps, in_maps, meta = prep_inputs(inputs)
        print("caps:", caps)
        nc = build_program(caps)
        sim = CoreSim(nc, trace=len(sys.argv) > 2)
        for k, v in in_maps[0].items():
            sim.tensor(k)[:] = v
        sim.simulate()
        print("sim time:", sim.time)
        o = np.asarray(sim.tensor("out"))
        rows, slot3 = meta[0]
        got0 = o[:, slot3].T
        exp0 = exp[rows]
        err = np.abs(got0 - exp0)
        print(f"sim core0 max abs err: {err.max():.3e}  "
              f"rel: {err.max()/np.abs(exp0).max():.3e}")

    else:
        got = kernel(**inputs)
        err = np.abs(got - exp)
        print(f"max abs err: {err.max():.3e}   "
              f"rel: {err.max()/np.abs(exp).max():.3e}")
